# revision 18
# baseline (speedup 1.0000x reference)
"""Trainium2 Bass kernel for an ensemble "differential layer":

    a    = W @ h + bias                          (N,B,OUT,1)
    hi   = tanh(a)
    g'   = 1 - hi^2 ;  g'' = -2 hi g'
    v    = W @ dh_dx                             (N,B,OUT,D)
    dhi_dx  = g' * v
    d2hi_d2x[i,o,j] = g''[o] v[o,i] v[o,j] + g'[o] * (W @ d2h_d2x[i,:,j])

Sharding: pure data parallel over (n, b). 8 cores; core c handles
n = c//2, batches b in [ (c%2)*64, (c%2)*64+64 ).

Device strategy:
  * Host pre-transposes the big tensors so every DMA moves >=2KiB
    contiguous runs per partition (line rate) with the contraction axis k
    (input) / output axis o on partitions:
        yt[b, k, i*32+j]    = d2h_d2x[n, b, i, k, j]
        out_t[b, o, i*32+j] = d2hi_d2x[n, b, i, o, j]
  * d2h_d2x / d2hi_d2x move as fp16 (halves DMA traffic; ~5e-4 rounding),
    the small matmul runs in float32r (~1.5e-4).
  * Rank-1 epilogue term is DVE-written into PSUM first; the two W@Y
    matmuls accumulate on top (start=False rides the pre-set has_written
    bits), and ScalarE evacuates with the per-partition g' scale:
        out = g' * (-2 hi * v (x) v  +  W @ Y)
  * hi / dhi_dx are accumulated on-chip across all 64 batches and stored
    once at the end (f32).
"""

import numpy as np

N, B, IN, OUT, D = 4, 128, 128, 128, 32
NCORES = 8
HALVES = 2              # cores per ensemble member
BPC = B // HALVES       # 64 batches per core
DD = D * D              # 1024

_prog_cache = {}

# test.py can flip these:
TRACE = False
TRACE_KWARGS = {}
LAST_RESULT = None      # BassKernelResults of the last run

# "fast"  : fp16 storage for d2h/d2hi + float32r small matmul (~1e-3 rel err)
# "f32r"  : f32 storage, float32r matmuls (~1.5e-4 rel err)
# "exact" : f32 storage, f32 matmuls (~1e-7 rel err)
PRECISION = "fast"


def _build_program(precision):
    import concourse.bacc as bacc
    import concourse.mybir as mybir
    from concourse import tile
    import bass_rust

    f32 = mybir.dt.float32
    if precision == "fast":
        ydt = mybir.dt.float16
    elif precision == "f32r":
        ydt = mybir.dt.float32r
    else:
        ydt = mybir.dt.float32
    mm1dt = f32  # small matmul: fp32r N=33 trips walrus fp32r restrictions
    alu = mybir.AluOpType
    act_fn = mybir.ActivationFunctionType

    nc = bacc.Bacc("TRN2", target_bir_lowering=False)

    CHUNK = 8                     # batches per DMA chunk
    NCH = BPC // CHUNK            # number of chunks
    CDD = CHUNK * DD              # elements per chunk row

    wty_d = nc.declare_dram_parameter("wty", [IN, OUT], ydt, isOutput=False)
    wt1_d = nc.declare_dram_parameter("wt1", [IN, OUT], mm1dt, isOutput=False)
    hx_d = nc.declare_dram_parameter("hx", [IN, BPC * 33], mm1dt, isOutput=False)
    yt_d = nc.declare_dram_parameter("yt", [NCH, IN, CDD], ydt, isOutput=False)
    bias_d = nc.declare_dram_parameter("bias", [OUT, 1], f32, isOutput=False)
    outt_d = nc.declare_dram_parameter("out_t", [NCH, OUT, CDD], ydt, isOutput=True)
    dhi_d = nc.declare_dram_parameter("dhi_t", [OUT, BPC * D], f32, isOutput=True)
    hi_d = nc.declare_dram_parameter("hi_t", [OUT, BPC], f32, isOutput=True)

    def bap(ap, dims):
        # manual AP on the same tensor: partition dim + given [step, count] dims
        return bass_rust.AP(ap.tensor, ap.offset, [list(ap.ap[0])] + dims)

    with tile.TileContext(nc) as tc:
        with (
            tc.tile_pool(name="const", bufs=1) as cpool,
            tc.tile_pool(name="y", bufs=4) as ypool,
            tc.tile_pool(name="o", bufs=3) as opool,
            tc.tile_pool(name="sm", bufs=4) as smpool,
        ):
            wty = cpool.tile([IN, OUT], ydt)
            nc.sync.dma_start(out=wty[:], in_=wty_d[:])
            wt1 = cpool.tile([IN, OUT], mm1dt)
            nc.sync.dma_start(out=wt1[:], in_=wt1_d[:])
            hx = cpool.tile([IN, BPC * 33], mm1dt)
            nc.sync.dma_start(out=hx[:], in_=hx_d[:])
            bias_t = cpool.tile([OUT, 1], f32)
            nc.sync.dma_start(out=bias_t[:], in_=bias_d[:])

            hi_acc = cpool.tile([OUT, BPC], f32)
            dhi_acc = cpool.tile([OUT, BPC * D], f32)

            # [a | v] for ALL batches up front: 5 matmuls -> SBUF (f32)
            AV = BPC * 33          # 2112
            pv_sb = cpool.tile([OUT, AV], f32)
            with tc.tile_pool(name="psv", bufs=2, space="PSUM") as psv:
                q0 = 0
                while q0 < AV:
                    qn = min(512, AV - q0)
                    pq = psv.tile([OUT, 512], f32, tag="pq", name=f"pq_{q0}")
                    nc.tensor.matmul(pq[:, 0:qn], wt1[:], hx[:, q0:q0 + qn])
                    nc.scalar.activation(pv_sb[:, q0:q0 + qn], pq[:, 0:qn],
                                         act_fn.Copy)
                    q0 += qn

            # Batched small ops over all 64 batches:
            #   hi = tanh(a + bias); g' = 1 - hi^2; va = hi*v; dhi = g'*v
            a_view = bap(pv_sb[:], [[33, BPC]])
            nc.scalar.activation(hi_acc[:], a_view, act_fn.Tanh,
                                 bias=bias_t[:], scale=1.0)
            hi2_all = cpool.tile([OUT, BPC], f32)
            nc.vector.tensor_tensor(hi2_all[:], hi_acc[:], hi_acc[:], alu.mult)
            gp_all = cpool.tile([OUT, BPC], f32)
            nc.vector.tensor_scalar(gp_all[:], hi2_all[:], -1.0, 1.0,
                                    alu.mult, alu.add)
            va_all = cpool.tile([OUT, BPC * D], f32)
            v_view = bass_rust.AP(pv_sb[:].tensor, pv_sb[:].offset + 1,
                                  [list(pv_sb[:].ap[0]), [33, BPC], [1, D]])
            va_view = bap(va_all[:], [[D, BPC], [1, D]])
            hi_bc = bap(hi_acc[:], [[1, BPC], [0, D]])
            nc.vector.tensor_tensor(va_view, v_view, hi_bc, alu.mult)
            dhi_view = bap(dhi_acc[:], [[D, BPC], [1, D]])
            gp_bc = bap(gp_all[:], [[1, BPC], [0, D]])
            nc.vector.tensor_tensor(dhi_view, v_view, gp_bc, alu.mult)

            # hi / dhi are final already -- store now, overlapping the loop
            nc.gpsimd.dma_start(out=dhi_d[:], in_=dhi_acc[:])
            nc.gpsimd.dma_start(out=hi_d[:], in_=hi_acc[:])

            # static PSUM quad buffer for the accumulate-on-top trick
            psu_cm = tc.tile_pool(name="psu", bufs=1, space="PSUM")
            psu = psu_cm.__enter__()
            NPU = 4
            pu_tiles = [psu.tile([OUT, DD], f32, tag=f"pu{i}", name=f"pu{i}")
                        for i in range(NPU)]
            # set has_written on all banks once (K=1 dummy matmuls);
            # values are overwritten later
            for pu in pu_tiles:
                nc.tensor.matmul(pu[:, 0:512], wt1[0:1, :], hx[0:1, 0:512],
                                 start=True, stop=True)
                nc.tensor.matmul(pu[:, 512:1024], wt1[0:1, :], hx[0:1, 0:512],
                                 start=True, stop=True)

            for cb in range(NCH):
                y_c = ypool.tile([IN, CDD], ydt, tag="y")
                nc.sync.dma_start(out=y_c[:], in_=yt_d[cb])
                ot_c = opool.tile([OUT, CDD], ydt, tag="ot")

                for s in range(CHUNK):
                    b = cb * CHUNK + s
                    pu = pu_tiles[b % NPU]

                    # PSUM <- (va*-2) (x) v  (rank-1 term, full overwrite)
                    pu_v = bap(pu[:], [[D, D], [1, D]])
                    va_b = bass_rust.AP(va_all[:].tensor,
                                        va_all[:].offset + b * D,
                                        [list(va_all[:].ap[0]), [1, D], [0, D]])
                    vb_b = bass_rust.AP(pv_sb[:].tensor,
                                        pv_sb[:].offset + b * 33 + 1,
                                        [list(pv_sb[:].ap[0]), [0, D], [1, D]])
                    nc.vector.scalar_tensor_tensor(pu_v, va_b, -2.0, vb_b,
                                                   alu.mult, alu.mult)

                    # PSUM += W @ Y  (accumulate onto DVE-written data)
                    y_b0 = s * DD
                    nc.tensor.matmul(pu[:, 0:512], wty[:],
                                     y_c[:, y_b0:y_b0 + 512],
                                     start=False, stop=True,
                                     skip_group_check=True)
                    nc.tensor.matmul(pu[:, 512:1024], wty[:],
                                     y_c[:, y_b0 + 512:y_b0 + 1024],
                                     start=False, stop=True,
                                     skip_group_check=True)

                    # out = g' * PSUM -> chunk staging
                    nc.scalar.activation(ot_c[:, y_b0:y_b0 + DD], pu[:],
                                         act_fn.Copy, bias=0.0,
                                         scale=gp_all[:, b:b + 1])

                nc.gpsimd.dma_start(out=outt_d[cb], in_=ot_c[:])

            psu_cm.__exit__(None, None, None)

    nc.compile()
    return nc


def _get_program(precision):
    if precision not in _prog_cache:
        _prog_cache[precision] = _build_program(precision)
    return _prog_cache[precision]


def kernel(h, dh_dx, d2h_d2x, weight, bias, hessian=1):
    global LAST_RESULT
    from concourse.bass_utils import run_bass_kernel_spmd

    h = np.asarray(h, np.float32)
    dh_dx = np.asarray(dh_dx, np.float32)
    d2h_d2x = np.asarray(d2h_d2x, np.float32)
    weight = np.asarray(weight, np.float32)
    bias_np = np.asarray(bias, np.float32)

    y_np_dtype = np.float16 if PRECISION == "fast" else np.float32

    nc = _get_program(PRECISION)

    in_maps = []
    for c in range(NCORES):
        n = c // HALVES
        b0 = (c % HALVES) * BPC
        wt = np.ascontiguousarray(weight[n, 0].T)                       # (IN, OUT)
        hxc = np.concatenate([h[n, b0:b0 + BPC], dh_dx[n, b0:b0 + BPC]], axis=-1)
        hxc = np.ascontiguousarray(hxc.transpose(1, 0, 2)).reshape(IN, BPC * 33)
        ytc = np.ascontiguousarray(
            d2h_d2x[n, b0:b0 + BPC].transpose(0, 2, 1, 3)
        ).reshape(BPC, IN, DD).astype(y_np_dtype)
        # chunk 8 batches per DMA: (8, IN, 8*DD) with batches side by side
        ytc = np.ascontiguousarray(
            ytc.reshape(8, 8, IN, DD).transpose(0, 2, 1, 3)
        ).reshape(8, IN, 8 * DD)
        in_maps.append({
            "wty": wt.astype(y_np_dtype), "wt1": wt, "hx": hxc, "yt": ytc,
            "bias": np.ascontiguousarray(bias_np[n, 0]),
        })

    res = run_bass_kernel_spmd(
        nc, in_maps, core_ids=list(range(NCORES)),
        trace=TRACE, trace_kwargs=TRACE_KWARGS,
    )
    LAST_RESULT = res

    hi = np.empty((N, B, OUT, 1), np.float32)
    dhi_dx = np.empty((N, B, OUT, D), np.float32)
    d2hi = np.empty((N, B, D, OUT, D), np.float32)
    for c in range(NCORES):
        n = c // HALVES
        b0 = (c % HALVES) * BPC
        r = res.results[c]
        hi[n, b0:b0 + BPC, :, 0] = r["hi_t"].T
        dhi_dx[n, b0:b0 + BPC] = r["dhi_t"].reshape(OUT, BPC, D).transpose(1, 0, 2)
        ot = r["out_t"].astype(np.float32).reshape(8, OUT, 8, DD).transpose(0, 2, 1, 3)
        d2hi[n, b0:b0 + BPC] = (
            ot.reshape(BPC, OUT, D, D).transpose(0, 2, 1, 3)
        )

    if not hessian:
        d2hi = d2h_d2x
    return hi, dhi_dx, d2hi


# revision 19
# speedup vs baseline: 1.0310x; 1.0310x over previous
"""Trainium2 Bass kernel for an ensemble "differential layer":

    a    = W @ h + bias                          (N,B,OUT,1)
    hi   = tanh(a)
    g'   = 1 - hi^2 ;  g'' = -2 hi g'
    v    = W @ dh_dx                             (N,B,OUT,D)
    dhi_dx  = g' * v
    d2hi_d2x[i,o,j] = g''[o] v[o,i] v[o,j] + g'[o] * (W @ d2h_d2x[i,:,j])

Sharding: pure data parallel over (n, b). 8 cores; core c handles
n = c//2, batches b in [ (c%2)*64, (c%2)*64+64 ).

Device strategy:
  * Host pre-transposes the big tensors so every DMA moves >=2KiB
    contiguous runs per partition (line rate) with the contraction axis k
    (input) / output axis o on partitions:
        yt[b, k, i*32+j]    = d2h_d2x[n, b, i, k, j]
        out_t[b, o, i*32+j] = d2hi_d2x[n, b, i, o, j]
  * d2h_d2x / d2hi_d2x move as fp16 (halves DMA traffic; ~5e-4 rounding),
    the small matmul runs in float32r (~1.5e-4).
  * Rank-1 epilogue term is DVE-written into PSUM first; the two W@Y
    matmuls accumulate on top (start=False rides the pre-set has_written
    bits), and ScalarE evacuates with the per-partition g' scale:
        out = g' * (-2 hi * v (x) v  +  W @ Y)
  * hi / dhi_dx are accumulated on-chip across all 64 batches and stored
    once at the end (f32).
"""

import numpy as np

N, B, IN, OUT, D = 4, 128, 128, 128, 32
NCORES = 8
HALVES = 2              # cores per ensemble member
BPC = B // HALVES       # 64 batches per core
DD = D * D              # 1024

_prog_cache = {}

# test.py can flip these:
TRACE = False
TRACE_KWARGS = {}
LAST_RESULT = None      # BassKernelResults of the last run

# "fast"  : fp16 storage for d2h/d2hi + float32r small matmul (~1e-3 rel err)
# "f32r"  : f32 storage, float32r matmuls (~1.5e-4 rel err)
# "exact" : f32 storage, f32 matmuls (~1e-7 rel err)
PRECISION = "fast"


def _build_program(precision):
    import concourse.bacc as bacc
    import concourse.mybir as mybir
    from concourse import tile
    import bass_rust

    f32 = mybir.dt.float32
    if precision == "fast":
        ydt = mybir.dt.float16
    elif precision == "f32r":
        ydt = mybir.dt.float32r
    else:
        ydt = mybir.dt.float32
    mm1dt = f32  # small matmul: fp32r N=33 trips walrus fp32r restrictions
    alu = mybir.AluOpType
    act_fn = mybir.ActivationFunctionType

    nc = bacc.Bacc("TRN2", target_bir_lowering=False)

    CHUNK = 8                     # batches per DMA chunk
    NCH = BPC // CHUNK            # number of chunks
    CDD = CHUNK * DD              # elements per chunk row

    wty_d = nc.declare_dram_parameter("wty", [IN, OUT], ydt, isOutput=False)
    wt1_d = nc.declare_dram_parameter("wt1", [IN, OUT], mm1dt, isOutput=False)
    hx_d = nc.declare_dram_parameter("hx", [IN, BPC * 33], mm1dt, isOutput=False)
    yt_d = nc.declare_dram_parameter("yt", [NCH, IN, CDD], ydt, isOutput=False)
    bias_d = nc.declare_dram_parameter("bias", [OUT, 1], f32, isOutput=False)
    outt_d = nc.declare_dram_parameter("out_t", [NCH, OUT, CDD], ydt, isOutput=True)
    dhi_d = nc.declare_dram_parameter("dhi_t", [OUT, BPC * D], f32, isOutput=True)
    hi_d = nc.declare_dram_parameter("hi_t", [OUT, BPC], f32, isOutput=True)

    def bap(ap, dims):
        # manual AP on the same tensor: partition dim + given [step, count] dims
        return bass_rust.AP(ap.tensor, ap.offset, [list(ap.ap[0])] + dims)

    with tile.TileContext(nc) as tc:
        with (
            tc.tile_pool(name="const", bufs=1) as cpool,
            tc.tile_pool(name="y", bufs=4) as ypool,
            tc.tile_pool(name="o", bufs=3) as opool,
            tc.tile_pool(name="sm", bufs=4) as smpool,
        ):
            wty = cpool.tile([IN, OUT], ydt)
            nc.sync.dma_start(out=wty[:], in_=wty_d[:])
            wt1 = cpool.tile([IN, OUT], mm1dt)
            nc.sync.dma_start(out=wt1[:], in_=wt1_d[:])
            hx = cpool.tile([IN, BPC * 33], mm1dt)
            nc.sync.dma_start(out=hx[:], in_=hx_d[:])
            bias_t = cpool.tile([OUT, 1], f32)
            nc.sync.dma_start(out=bias_t[:], in_=bias_d[:])

            hi_acc = cpool.tile([OUT, BPC], f32)
            dhi_acc = cpool.tile([OUT, BPC * D], f32)

            # [a | v] for ALL batches up front: 5 matmuls -> SBUF (f32)
            AV = BPC * 33          # 2112
            pv_sb = cpool.tile([OUT, AV], f32)
            with tc.tile_pool(name="psv", bufs=2, space="PSUM") as psv:
                q0 = 0
                while q0 < AV:
                    qn = min(512, AV - q0)
                    pq = psv.tile([OUT, 512], f32, tag="pq", name=f"pq_{q0}")
                    nc.tensor.matmul(pq[:, 0:qn], wt1[:], hx[:, q0:q0 + qn])
                    nc.scalar.activation(pv_sb[:, q0:q0 + qn], pq[:, 0:qn],
                                         act_fn.Copy)
                    q0 += qn

            # Batched small ops over all 64 batches:
            #   hi = tanh(a + bias); g' = 1 - hi^2; va = hi*v; dhi = g'*v
            a_view = bap(pv_sb[:], [[33, BPC]])
            nc.scalar.activation(hi_acc[:], a_view, act_fn.Tanh,
                                 bias=bias_t[:], scale=1.0)
            hi2_all = cpool.tile([OUT, BPC], f32)
            nc.vector.tensor_tensor(hi2_all[:], hi_acc[:], hi_acc[:], alu.mult)
            gp_all = cpool.tile([OUT, BPC], f32)
            nc.vector.tensor_scalar(gp_all[:], hi2_all[:], -1.0, 1.0,
                                    alu.mult, alu.add)
            va_all = cpool.tile([OUT, BPC * D], f32)
            v_view = bass_rust.AP(pv_sb[:].tensor, pv_sb[:].offset + 1,
                                  [list(pv_sb[:].ap[0]), [33, BPC], [1, D]])
            va_view = bap(va_all[:], [[D, BPC], [1, D]])
            hi_bc = bap(hi_acc[:], [[1, BPC], [0, D]])
            nc.vector.tensor_tensor(va_view, v_view, hi_bc, alu.mult)
            dhi_view = bap(dhi_acc[:], [[D, BPC], [1, D]])
            gp_bc = bap(gp_all[:], [[1, BPC], [0, D]])
            nc.vector.tensor_tensor(dhi_view, v_view, gp_bc, alu.mult)

            # static PSUM quad buffer for the accumulate-on-top trick
            psu_cm = tc.tile_pool(name="psu", bufs=1, space="PSUM")
            psu = psu_cm.__enter__()
            NPU = 4
            pu_tiles = [psu.tile([OUT, DD], f32, tag=f"pu{i}", name=f"pu{i}")
                        for i in range(NPU)]
            # set has_written on all banks once (K=1 dummy matmuls);
            # values are overwritten later
            for pu in pu_tiles:
                nc.tensor.matmul(pu[:, 0:512], wt1[0:1, :], hx[0:1, 0:512],
                                 start=True, stop=True)
                nc.tensor.matmul(pu[:, 512:1024], wt1[0:1, :], hx[0:1, 0:512],
                                 start=True, stop=True)

            for cb in range(NCH):
                y_c = ypool.tile([IN, CDD], ydt, tag="y")
                nc.sync.dma_start(out=y_c[:], in_=yt_d[cb])
                ot_c = opool.tile([OUT, CDD], ydt, tag="ot")

                for s in range(CHUNK):
                    b = cb * CHUNK + s
                    pu = pu_tiles[b % NPU]

                    # PSUM <- (va*-2) (x) v  (rank-1 term, full overwrite)
                    pu_v = bap(pu[:], [[D, D], [1, D]])
                    va_b = bass_rust.AP(va_all[:].tensor,
                                        va_all[:].offset + b * D,
                                        [list(va_all[:].ap[0]), [1, D], [0, D]])
                    vb_b = bass_rust.AP(pv_sb[:].tensor,
                                        pv_sb[:].offset + b * 33 + 1,
                                        [list(pv_sb[:].ap[0]), [0, D], [1, D]])
                    nc.vector.scalar_tensor_tensor(pu_v, va_b, -2.0, vb_b,
                                                   alu.mult, alu.mult)

                    # PSUM += W @ Y  (accumulate onto DVE-written data)
                    y_b0 = s * DD
                    nc.tensor.matmul(pu[:, 0:512], wty[:],
                                     y_c[:, y_b0:y_b0 + 512],
                                     start=False, stop=True,
                                     skip_group_check=True)
                    nc.tensor.matmul(pu[:, 512:1024], wty[:],
                                     y_c[:, y_b0 + 512:y_b0 + 1024],
                                     start=False, stop=True,
                                     skip_group_check=True)

                    # out = g' * PSUM -> chunk staging
                    nc.scalar.activation(ot_c[:, y_b0:y_b0 + DD], pu[:],
                                         act_fn.Copy, bias=0.0,
                                         scale=gp_all[:, b:b + 1])

                nc.gpsimd.dma_start(out=outt_d[cb], in_=ot_c[:])

            nc.gpsimd.dma_start(out=dhi_d[:], in_=dhi_acc[:])
            nc.gpsimd.dma_start(out=hi_d[:], in_=hi_acc[:])
            psu_cm.__exit__(None, None, None)

    nc.compile()
    return nc


def _get_program(precision):
    if precision not in _prog_cache:
        _prog_cache[precision] = _build_program(precision)
    return _prog_cache[precision]


def kernel(h, dh_dx, d2h_d2x, weight, bias, hessian=1):
    global LAST_RESULT
    from concourse.bass_utils import run_bass_kernel_spmd

    h = np.asarray(h, np.float32)
    dh_dx = np.asarray(dh_dx, np.float32)
    d2h_d2x = np.asarray(d2h_d2x, np.float32)
    weight = np.asarray(weight, np.float32)
    bias_np = np.asarray(bias, np.float32)

    y_np_dtype = np.float16 if PRECISION == "fast" else np.float32

    nc = _get_program(PRECISION)

    in_maps = []
    for c in range(NCORES):
        n = c // HALVES
        b0 = (c % HALVES) * BPC
        wt = np.ascontiguousarray(weight[n, 0].T)                       # (IN, OUT)
        hxc = np.concatenate([h[n, b0:b0 + BPC], dh_dx[n, b0:b0 + BPC]], axis=-1)
        hxc = np.ascontiguousarray(hxc.transpose(1, 0, 2)).reshape(IN, BPC * 33)
        ytc = np.ascontiguousarray(
            d2h_d2x[n, b0:b0 + BPC].transpose(0, 2, 1, 3)
        ).reshape(BPC, IN, DD).astype(y_np_dtype)
        # chunk 8 batches per DMA: (8, IN, 8*DD) with batches side by side
        ytc = np.ascontiguousarray(
            ytc.reshape(8, 8, IN, DD).transpose(0, 2, 1, 3)
        ).reshape(8, IN, 8 * DD)
        in_maps.append({
            "wty": wt.astype(y_np_dtype), "wt1": wt, "hx": hxc, "yt": ytc,
            "bias": np.ascontiguousarray(bias_np[n, 0]),
        })

    res = run_bass_kernel_spmd(
        nc, in_maps, core_ids=list(range(NCORES)),
        trace=TRACE, trace_kwargs=TRACE_KWARGS,
    )
    LAST_RESULT = res

    hi = np.empty((N, B, OUT, 1), np.float32)
    dhi_dx = np.empty((N, B, OUT, D), np.float32)
    d2hi = np.empty((N, B, D, OUT, D), np.float32)
    for c in range(NCORES):
        n = c // HALVES
        b0 = (c % HALVES) * BPC
        r = res.results[c]
        hi[n, b0:b0 + BPC, :, 0] = r["hi_t"].T
        dhi_dx[n, b0:b0 + BPC] = r["dhi_t"].reshape(OUT, BPC, D).transpose(1, 0, 2)
        ot = r["out_t"].astype(np.float32).reshape(8, OUT, 8, DD).transpose(0, 2, 1, 3)
        d2hi[n, b0:b0 + BPC] = (
            ot.reshape(BPC, OUT, D, D).transpose(0, 2, 1, 3)
        )

    if not hessian:
        d2hi = d2h_d2x
    return hi, dhi_dx, d2hi
